# revision 1
# baseline (speedup 1.0000x reference)
"""GCN layer (gather -> mean-aggregate -> linear) on 8 Trainium2 cores.

Strategy (SPMD, no collectives):
  - Nodes are row-sharded: core c owns dst nodes [c*S, (c+1)*S), S = N/8.
  - Edges are bucketed by dst-owner core on the host and turned into a dense
    per-core adjacency count matrix A[src_node, local_dst] (fp8e4m3 - counts
    are small integers, exact). The per-core segment sum is then a dense
    GEMM on the PE array:  sums = A.T-blocks @ x, with x shipped as an exact
    bf16 hi/lo split table [bf16(x) | 1 | bf16(x - bf16(x))] so products are
    f32-accurate and the in-degree falls out of the ones column for free.
  - PSUM: matmul start=True zeroes a whole 2KB bank, so each of the <=8
    concurrently-accumulating node groups owns one bank; 10 groups run as
    passes of 8 + 2.
  - Phase 3 per 128-node tile: h = a*x + b*sums (a,b from degree), PE f32
    transpose of h, out = hT.T @ W + bias, row-sharded output gathered on
    the host.
"""

import os

import numpy as np

CORES = 8
TRACE = False           # set by test harness to print HW exec time
_cache = {}


def _build_program(N, F, FO, R):
    from concourse import bacc, tile
    from concourse.bass import mybir

    F32 = mybir.dt.float32
    BF16 = mybir.dt.bfloat16
    FP8 = mybir.dt.float8e4
    hi_lo = True
    KT = (N + 127) // 128          # K slabs
    NT = R // 128                  # node groups per core
    assert NT <= 16
    nc = bacc.Bacc(None)

    xtabd = nc.dram_tensor("xtab", [128, KT * 260], BF16, kind="ExternalInput")
    A = nc.dram_tensor("A", [KT * 128, R], FP8, kind="ExternalInput")
    xs = nc.dram_tensor("xs", [R, F], F32, kind="ExternalInput")
    Wt = nc.dram_tensor("W", [F, FO], F32, kind="ExternalInput")
    bt = nc.dram_tensor("b", [128, FO], F32, kind="ExternalInput")
    identd = nc.dram_tensor("ident", [128, 128], F32, kind="ExternalInput")
    out = nc.dram_tensor("out", [R, FO], F32, kind="ExternalOutput")

    # matmul start=True zeroes a whole 2KB PSUM bank, so each live
    # accumulation group owns a bank (max 8). Bank map:
    #   pass A (groups 0..7):  banks 0..7      pass B (8..NT): banks 0..1
    #   phase 3: out psum banks 2/3 (alternating), hT psum banks 4/5
    psall = nc.alloc_psum_tensor("psall", [128, 4096], F32)

    with tile.TileContext(nc) as tc:
        with (
            tc.tile_pool(name="const", bufs=1) as cpool,
            tc.tile_pool(name="xload", bufs=4) as xpool,
            tc.tile_pool(name="acc", bufs=1) as accpool,
            tc.tile_pool(name="p3", bufs=4) as p3pool,
        ):
            # constants on the scalar HWDGE queue so the sync queue's head
            # is free for the critical first xtab/A chunks
            wt_sb = cpool.tile([128, FO], F32, name="wt_sb")
            nc.scalar.dma_start(wt_sb[:], Wt[:])
            b_sb = cpool.tile([128, FO], F32, name="b_sb")
            nc.scalar.dma_start(b_sb[:], bt[:])
            ident = cpool.tile([128, 128], F32, name="ident")
            nc.scalar.dma_start(ident[:], identd[:])

            # PE warm-up: ~3us of tiny matmuls run during the first-chunk
            # DMA wait so the HAM clock gate is at full rate when the real
            # stream starts (first-80-mm avg was 173ns vs 110ns steady).
            # They write a phase-3 psum bank, whose first real use re-inits
            # with start=True.
            warm = cpool.tile([128, 128], BF16, name="warm")
            nc.vector.memset(warm[:], 0.0)
            for _w in range(40):
                nc.tensor.matmul(
                    psall[:16, 1024:1152], warm[:, 0:16], warm[:, 0:128],
                    start=True, stop=True, skip_group_check=True,
                )

            # ---- phase 0: x table [xhi | 1 | xlo] (bf16), host-prepared;
            # loaded in 4-slab chunks interleaved with the first pass ----
            NW = 257 if hi_lo else 129
            xtab = accpool.tile([128, KT, 260], BF16, name="xtab", tag="xtab")
            xtab_flat = xtab[:].rearrange("p a b -> p (a b)")

            def load_xtab_chunk(k0, k1):
                c0 = k0 * 260
                c1 = min(KT, k1) * 260
                nc.sync.dma_start(xtab_flat[:, c0:c1], xtabd[:, c0:c1])

            # ---- phase 1: adjacency matmuls, passes of <=8 groups ----
            sums_sb = accpool.tile([128, NT, F], F32)
            deg_sb = accpool.tile([128, NT], F32)

            # A resident in SBUF: [128, KT, R] fp8, loaded in 4-slab strided
            # chunks interleaved with the first pass (row 128k+p -> [p, k, :])
            A_sb = accpool.tile([128, KT, R], FP8, name="A_sb", tag="A_sb")
            NCH = (KT + 3) // 4

            def load_a_chunk(k0, k1, j):
                k1 = min(KT, k1)
                src_ap = A[128 * k0 : 128 * k1, :].rearrange(
                    "(k p) c -> p k c", p=128
                )
                deng = nc.scalar if j % 2 else nc.sync
                deng.dma_start(A_sb[:, k0:k1, :], src_ap)

            # chunk schedule: tiny first chunks so matmul 0 starts early,
            # then 4-slab chunks prefetched one ahead of the consume point
            bounds = [0, 1, 2] + list(range(5, KT, 4)) + [KT]
            chunks = list(zip(bounds, bounds[1:]))

            for gi, g0 in enumerate(range(0, NT, 8)):
                g1 = min(NT, g0 + 8)
                if gi == 0:
                    load_xtab_chunk(*chunks[0])
                    load_a_chunk(*chunks[0], 0)
                    nxt = 1
                for k in range(KT):
                    if gi == 0:
                        while nxt < len(chunks) and chunks[nxt][0] <= k + 2:
                            load_xtab_chunk(*chunks[nxt])
                            load_a_chunk(*chunks[nxt], nxt)
                            nxt += 1
                    st = k == 0
                    sp = k == KT - 1
                    for g in range(g0, g1):
                        lhs = A_sb[:, k, 128 * g : 128 * (g + 1)]
                        ps = psall[:, 512 * (g - g0) : 512 * (g - g0) + NW]
                        nc.tensor.matmul(
                            ps, lhs, xtab[:, k, 0:NW],
                            start=st, stop=sp, skip_group_check=False,
                        )
                for g in range(g0, g1):
                    ps = psall[:, 512 * (g - g0) : 512 * (g - g0) + NW]
                    nc.scalar.copy(sums_sb[:, g, :], ps[:, 0:128])
                    if hi_lo:
                        nc.vector.tensor_add(sums_sb[:, g, :], sums_sb[:, g, :],
                                             ps[:, 129:257])
                    nc.scalar.copy(deg_sb[:, g : g + 1], ps[:, 128:129])

            # ---- phase 3 ----
            # per-node coefficients, one tile at a time (a whole-deg_sb op
            # would make tile 0 wait for the LAST pass's flush):
            #   h = a*x + bb*sums,  a = 1-min(deg,1), bb = min(deg,1)/max(deg,1)
            a_all = accpool.tile([128, NT], F32)
            bb_all = accpool.tile([128, NT], F32)

            for t in range(NT):
                rows = slice(128 * t, 128 * (t + 1))
                ps3 = psall[:, 1024 + (t % 3) * 512 : 1536 + (t % 3) * 512]
                psT = psall[:, 2560 + (t % 3) * 512 : 2688 + (t % 3) * 512]
                xt = p3pool.tile([128, F], F32, tag="xt")
                nc.sync.dma_start(xt[:], xs[rows, :])

                dcol = deg_sb[:, t : t + 1]
                acol = a_all[:, t : t + 1]
                bcol = bb_all[:, t : t + 1]
                rec = p3pool.tile([128, 1], F32, tag="rec")
                nc.vector.tensor_scalar_max(rec[:], dcol, 1.0)
                nc.vector.reciprocal(rec[:], rec[:])
                nc.vector.tensor_scalar_min(bcol, dcol, 1.0)
                nc.vector.tensor_scalar(acol, bcol, -1.0, None,
                                        op0=mybir.AluOpType.mult)
                nc.vector.tensor_scalar_add(acol, acol, 1.0)
                nc.vector.tensor_mul(bcol, bcol, rec[:])

                h = p3pool.tile([128, F], F32, tag="h")
                tmp = p3pool.tile([128, F], F32, tag="tmp")
                nc.scalar.mul(tmp[:], sums_sb[:, t, :], bcol)
                nc.vector.scalar_tensor_tensor(
                    h[:], xt[:], acol, tmp[:],
                    op0=mybir.AluOpType.mult, op1=mybir.AluOpType.add,
                )

                nc.tensor.transpose(psT, h[:], ident[:])             # PE f32
                hTs = p3pool.tile([128, F], F32, tag="hTs")
                nc.scalar.copy(hTs[:], psT)

                nc.tensor.matmul(ps3, hTs[:], wt_sb[:], start=True, stop=True,
                                 skip_group_check=True)
                ot = p3pool.tile([128, FO], F32, tag="ot")
                nc.vector.tensor_add(ot[:], b_sb[:], ps3)
                nc.sync.dma_start(out[rows, :], ot[:])

    nc.compile()
    return nc


def _make_xtab(x32, KT):
    import ml_dtypes

    N, F = x32.shape
    xt = np.zeros((128, KT, 260), dtype=ml_dtypes.bfloat16)
    xf = np.zeros((KT * 128, F), np.float32)
    xf[:N] = x32
    xf = xf.reshape(KT, 128, F).transpose(1, 0, 2)
    hi = xf.astype(ml_dtypes.bfloat16)
    xt[:, :, 0:128] = hi
    xt[:, :, 128] = 1.0
    xt[:, :, 129:257] = (xf - hi.astype(np.float32)).astype(ml_dtypes.bfloat16)
    return np.ascontiguousarray(xt.reshape(128, KT * 260))


def _shard_inputs(x32, src, dst, W32, b32, n_cores):
    import ml_dtypes

    N, F = x32.shape
    S = (N + n_cores - 1) // n_cores
    NT = (S + 127) // 128
    R = NT * 128
    KT = (N + 127) // 128
    owner = np.minimum(dst // S, n_cores - 1)
    xtab = _make_xtab(x32, KT)
    brep = np.ascontiguousarray(np.tile(b32.reshape(1, -1), (128, 1)))
    ident = np.eye(128, dtype=np.float32)
    in_maps = []
    for c in range(n_cores):
        sel = owner == c
        A = np.zeros((KT * 128, R), np.float32)
        np.add.at(A, (src[sel], dst[sel] - c * S), 1.0)
        assert A.max() <= 16, "edge multiplicity too large for fp8e4m3"
        xs = np.zeros((R, F), dtype=np.float32)
        lo = c * S
        hi = min(N, lo + S)
        xs[: hi - lo] = x32[lo:hi]
        in_maps.append(
            {
                "xtab": xtab,
                "A": A.astype(ml_dtypes.float8_e4m3),
                "xs": xs,
                "W": W32,
                "b": brep,
                "ident": ident,
            }
        )
    return in_maps, R


def _install_ntff_shim():
    """antenv.axon_hooks shim so trace=True can NTFF-profile in this env."""
    import contextlib
    import ctypes
    import sys
    import types

    if "antenv.axon_hooks" in sys.modules:
        return
    so_path = "/opt/axon/libaxon_pjrt.so"
    try:
        lib = ctypes.CDLL(so_path)
        lib.axon_start_nrt_profile.argtypes = [
            ctypes.POINTER(ctypes.c_int64), ctypes.c_size_t]
        lib.axon_start_nrt_profile.restype = ctypes.c_int64
        lib.axon_stop_nrt_profile.argtypes = [ctypes.c_char_p]
        lib.axon_stop_nrt_profile.restype = ctypes.c_int64
    except Exception:
        return

    @contextlib.contextmanager
    def _hook(output_dir, device_ids):
        import jax

        jax.devices()
        if device_ids:
            ids = (ctypes.c_int64 * len(device_ids))(*device_ids)
            rc = lib.axon_start_nrt_profile(ids, len(device_ids))
        else:
            rc = lib.axon_start_nrt_profile(None, 0)
        if rc != 0:
            raise RuntimeError(f"axon_start_nrt_profile rc={rc}")
        try:
            yield
        finally:
            lib.axon_stop_nrt_profile(str(output_dir).encode())

    mod = types.ModuleType("antenv.axon_hooks")
    mod.set_axon_ntff_profile_hook = lambda h: None
    mod.get_axon_ntff_profile_hook = lambda: _hook
    sys.modules["antenv.axon_hooks"] = mod


def kernel(x, src, dst, W, b):
    from concourse import bass_utils

    x32 = np.ascontiguousarray(np.asarray(x), dtype=np.float32)
    W32 = np.ascontiguousarray(np.asarray(W), dtype=np.float32)
    b32 = np.ascontiguousarray(np.asarray(b), dtype=np.float32)
    src = np.asarray(src).astype(np.int64)
    dst = np.asarray(dst).astype(np.int64)
    N, F = x32.shape
    FO = W32.shape[1]
    S = (N + CORES - 1) // CORES

    in_maps, R = _shard_inputs(x32, src, dst, W32, b32, CORES)

    key = (N, F, FO, R)
    if key not in _cache:
        _cache[key] = _build_program(N, F, FO, R)
    nc = _cache[key]

    if TRACE:
        _install_ntff_shim()

    last_err = None
    for _attempt in range(2):
        try:
            res = bass_utils.run_bass_kernel_spmd(
                nc, in_maps, core_ids=list(range(CORES)), trace=TRACE
            )
            break
        except Exception as e:  # retry once on transient device errors
            last_err = e
    else:
        raise last_err

    if TRACE and res.exec_time_ns is not None:
        print("HW exec time:", res.exec_time_ns, "ns")

    outs = [np.asarray(r["out"]).reshape(R, FO) for r in res.results]
    full = np.concatenate([o[:S] for o in outs], axis=0)[:N]
    return full.astype(np.float32)



# revision 3
# speedup vs baseline: 1.6000x; 1.6000x over previous
"""GCN layer (gather -> mean-aggregate -> linear) on 8 Trainium2 cores.

Strategy (SPMD, no collectives):
  - Nodes are row-sharded: core c owns dst nodes [c*S, (c+1)*S), S = N/8.
  - Edges are bucketed by dst-owner core on the host and turned into a dense
    per-core adjacency count matrix A[src_node, local_dst] (fp8e4m3 - counts
    are small integers, exact). The per-core segment sum is computed
    FEATURE-MAJOR on the PE array:  sumsT = x.T @ A, with x as bf16 slabs
    (lhsT stationary) and A streaming as the rhs.  This streams R=1280
    columns per src slab (vs 2*F+1 per slab per node-group for the
    node-major orientation) - half the PE cycles.
  - Degrees (and the zero-in-degree fallback) are folded in on the host:
    rb = 1/max(deg,1) is shipped per node, and zero-degree nodes get a
    self-edge in A so mean==x for them (DGL recv semantics), making the
    device program branch-free.
  - Phase 3 per 128-node tile: out = (sumsT_tile.T @ W) * rb + b.  sumsT is
    already the lhsT layout the GEMM wants - no PE transposes at all.
"""

import os

import numpy as np

CORES = 8
TRACE = False           # set by test harness to print HW exec time
_cache = {}


def _build_program(N, F, FO, R):
    from concourse import bacc, tile
    from concourse.bass import mybir

    F32 = mybir.dt.float32
    BF16 = mybir.dt.bfloat16
    FP8 = mybir.dt.float8e4
    KT = (N + 127) // 128          # src-node slabs
    NT = R // 128                  # owned-node tiles per core
    nc = bacc.Bacc(None)

    xtd = nc.dram_tensor("xt", [128, KT * F], BF16, kind="ExternalInput")
    Ad = nc.dram_tensor("A", [128, KT * R], FP8, kind="ExternalInput")
    Wd = nc.dram_tensor("W", [F, FO], BF16, kind="ExternalInput")
    bd = nc.dram_tensor("b", [128, FO], F32, kind="ExternalInput")
    rbd = nc.dram_tensor("rb", [128, NT], F32, kind="ExternalInput")
    out = nc.dram_tensor("out", [R, FO], F32, kind="ExternalOutput")

    # PSUM bank map (each accumulating group owns a 2KB bank, zeroed by
    # its first start=True matmul):
    #   banks 0..2: phase-1 sumsT col groups [0:512],[512:1024],[1024:1280]
    #   banks 3..5: phase-3 out psum, cycling t%3
    #   bank 7 head: PE warm-up scratch
    psall = nc.alloc_psum_tensor("psall", [128, 4096], F32)

    cgrps = [(c0, min(R, c0 + 512)) for c0 in range(0, R, 512)]

    with tile.TileContext(nc) as tc:
        with (
            tc.tile_pool(name="const", bufs=1) as cpool,
            tc.tile_pool(name="acc", bufs=1) as accpool,
            tc.tile_pool(name="p3", bufs=4) as p3pool,
        ):
            # constants on the scalar HWDGE queue so the sync queue's head
            # is free for the critical first x/A chunks
            wt_sb = cpool.tile([128, FO], BF16, name="wt_sb")
            nc.scalar.dma_start(wt_sb[:], Wd[:])
            b_sb = cpool.tile([128, FO], F32, name="b_sb")
            nc.scalar.dma_start(b_sb[:], bd[:])
            rb_sb = cpool.tile([128, NT], F32, name="rb_sb")
            nc.scalar.dma_start(rb_sb[:], rbd[:])

            # PE warm-up: ~3us of tiny matmuls run during the first-chunk
            # DMA wait so the HAM clock gate is at full rate when the real
            # stream starts.
            warm = cpool.tile([128, 128], BF16, name="warm")
            nc.vector.memset(warm[:], 0.0)
            for _w in range(40):
                nc.tensor.matmul(
                    psall[:16, 3584:3712], warm[:, 0:16], warm[:, 0:128],
                    start=True, stop=True, skip_group_check=True,
                )

            # ---- SBUF-resident inputs, loaded in chunks interleaved with
            # the phase-1 slab loop ----
            x_sb = accpool.tile([128, KT, F], BF16, name="x_sb", tag="x_sb")
            x_flat = x_sb[:].rearrange("p a b -> p (a b)")
            A_sb = accpool.tile([128, KT, R], FP8, name="A_sb", tag="A_sb")
            A_flat = A_sb[:].rearrange("p a b -> p (a b)")
            sumsT = accpool.tile([128, R], BF16, name="sumsT")

            def load_x_chunk(k0, k1, j):
                deng = nc.scalar if j % 2 else nc.sync
                deng.dma_start(x_flat[:, k0 * F : k1 * F], xtd[:, k0 * F : k1 * F])

            def load_a_chunk(k0, k1, j):
                deng = nc.scalar if j % 2 else nc.sync
                deng.dma_start(A_flat[:, k0 * R : k1 * R], Ad[:, k0 * R : k1 * R])

            # chunk schedule: tiny first chunks so matmul 0 starts early,
            # then big chunks prefetched ahead of the consume point
            ba = [0, 1, 2, 3, 5, 7, 11, 15]
            while ba[-1] < KT:
                ba.append(min(KT, ba[-1] + 8))
            a_chunks = list(zip(ba, ba[1:]))
            bx = [0, 2, 4, 8, 16, 28]
            while bx[-1] < KT:
                bx.append(min(KT, bx[-1] + 16))
            x_chunks = list(zip(bx, bx[1:]))

            load_x_chunk(*x_chunks[0], 0)
            load_a_chunk(*a_chunks[0], 1)
            nxa, nxx = 1, 1

            # ---- phase 1: sumsT[f, d] += x[s, f] * A[s, d], slab-major ----
            for k in range(KT):
                while nxx < len(x_chunks) and x_chunks[nxx][0] <= k + 6:
                    load_x_chunk(*x_chunks[nxx], nxx)
                    nxx += 1
                while nxa < len(a_chunks) and a_chunks[nxa][0] <= k + 3:
                    load_a_chunk(*a_chunks[nxa], nxa)
                    nxa += 1
                st = k == 0
                sp = k == KT - 1
                for (c0, c1) in cgrps:
                    nc.tensor.matmul(
                        psall[:, c0:c1], x_sb[:, k, :], A_sb[:, k, c0:c1],
                        start=st, stop=sp, skip_group_check=False,
                    )

            # ---- phase 2: evacuate PSUM -> SBUF (cast to bf16) ----
            nc.scalar.copy(sumsT[:, cgrps[0][0]:cgrps[0][1]],
                           psall[:, cgrps[0][0]:cgrps[0][1]])
            nc.vector.tensor_scalar_mul(sumsT[:, cgrps[1][0]:cgrps[1][1]],
                                        psall[:, cgrps[1][0]:cgrps[1][1]], 1.0)
            nc.scalar.copy(sumsT[:, cgrps[2][0]:cgrps[2][1]],
                           psall[:, cgrps[2][0]:cgrps[2][1]])

            # ---- phase 3: per node tile, out = (sumsT_t.T @ W) * rb + b ----
            for t in range(NT):
                rows = slice(128 * t, 128 * (t + 1))
                ps3 = psall[:, 1536 + (t % 3) * 512 : 2048 + (t % 3) * 512]
                nc.tensor.matmul(ps3, sumsT[:, rows], wt_sb[:],
                                 start=True, stop=True, skip_group_check=True)
                ot = p3pool.tile([128, FO], F32, tag="ot")
                nc.vector.scalar_tensor_tensor(
                    ot[:], ps3, rb_sb[:, t : t + 1], b_sb[:],
                    op0=mybir.AluOpType.mult, op1=mybir.AluOpType.add,
                )
                deng = nc.scalar if t % 2 else nc.sync
                deng.dma_start(out[rows, :], ot[:])

    nc.compile()
    return nc


def _shard_inputs(x32, src, dst, W32, b32, n_cores):
    import ml_dtypes

    N, F = x32.shape
    FO = W32.shape[1]
    S = (N + n_cores - 1) // n_cores
    NT = (S + 127) // 128
    R = NT * 128
    KT = (N + 127) // 128

    # x slabs, feature-minor: xt[p, k, f] = x[128k + p, f], bf16
    xp = np.zeros((KT * 128, F), np.float32)
    xp[:N] = x32
    xt = np.ascontiguousarray(
        xp.reshape(KT, 128, F).transpose(1, 0, 2).astype(ml_dtypes.bfloat16)
    ).reshape(128, KT * F)

    deg = np.bincount(dst, minlength=N)
    rb_full = (1.0 / np.maximum(deg, 1)).astype(np.float32)
    zero_nodes = np.where(deg == 0)[0]

    brep = np.ascontiguousarray(np.tile(b32.reshape(1, -1), (128, 1)))
    Wb = W32.astype(ml_dtypes.bfloat16)

    in_maps = []
    for c in range(n_cores):
        lo = c * S
        hi = min(N, lo + S)
        sel = (dst >= lo) & (dst < hi)
        A = np.zeros((KT * 128, R), np.float32)
        np.add.at(A, (src[sel], dst[sel] - lo), 1.0)
        zn = zero_nodes[(zero_nodes >= lo) & (zero_nodes < hi)]
        if zn.size:  # self-edge: zero-in-degree nodes keep their input
            A[zn, zn - lo] += 1.0
        assert A.max() <= 16, "edge multiplicity too large for fp8e4m3"
        A8 = np.ascontiguousarray(
            A.reshape(KT, 128, R).transpose(1, 0, 2).astype(ml_dtypes.float8_e4m3)
        ).reshape(128, KT * R)
        rb_c = np.ones(R, np.float32)
        rb_c[: hi - lo] = rb_full[lo:hi]
        rb_c = np.ascontiguousarray(rb_c.reshape(NT, 128).T)
        in_maps.append({"xt": xt, "A": A8, "W": Wb, "b": brep, "rb": rb_c})
    return in_maps, R


def _install_ntff_shim():
    """antenv.axon_hooks shim so trace=True can NTFF-profile in this env."""
    import contextlib
    import ctypes
    import sys
    import types

    if "antenv.axon_hooks" in sys.modules:
        return
    so_path = "/opt/axon/libaxon_pjrt.so"
    try:
        lib = ctypes.CDLL(so_path)
        lib.axon_start_nrt_profile.argtypes = [
            ctypes.POINTER(ctypes.c_int64), ctypes.c_size_t]
        lib.axon_start_nrt_profile.restype = ctypes.c_int64
        lib.axon_stop_nrt_profile.argtypes = [ctypes.c_char_p]
        lib.axon_stop_nrt_profile.restype = ctypes.c_int64
    except Exception:
        return

    @contextlib.contextmanager
    def _hook(output_dir, device_ids):
        import jax

        jax.devices()
        if device_ids:
            ids = (ctypes.c_int64 * len(device_ids))(*device_ids)
            rc = lib.axon_start_nrt_profile(ids, len(device_ids))
        else:
            rc = lib.axon_start_nrt_profile(None, 0)
        if rc != 0:
            raise RuntimeError(f"axon_start_nrt_profile rc={rc}")
        try:
            yield
        finally:
            lib.axon_stop_nrt_profile(str(output_dir).encode())

    mod = types.ModuleType("antenv.axon_hooks")
    mod.set_axon_ntff_profile_hook = lambda h: None
    mod.get_axon_ntff_profile_hook = lambda: _hook
    sys.modules["antenv.axon_hooks"] = mod


def kernel(x, src, dst, W, b):
    from concourse import bass_utils

    x32 = np.ascontiguousarray(np.asarray(x), dtype=np.float32)
    W32 = np.ascontiguousarray(np.asarray(W), dtype=np.float32)
    b32 = np.ascontiguousarray(np.asarray(b), dtype=np.float32)
    src = np.asarray(src).astype(np.int64)
    dst = np.asarray(dst).astype(np.int64)
    N, F = x32.shape
    FO = W32.shape[1]
    S = (N + CORES - 1) // CORES

    in_maps, R = _shard_inputs(x32, src, dst, W32, b32, CORES)

    key = (N, F, FO, R)
    if key not in _cache:
        _cache[key] = _build_program(N, F, FO, R)
    nc = _cache[key]

    if TRACE:
        _install_ntff_shim()

    last_err = None
    for _attempt in range(2):
        try:
            res = bass_utils.run_bass_kernel_spmd(
                nc, in_maps, core_ids=list(range(CORES)), trace=TRACE
            )
            break
        except Exception as e:  # retry once on transient device errors
            last_err = e
    else:
        raise last_err

    if TRACE and res.exec_time_ns is not None:
        print("HW exec time:", res.exec_time_ns, "ns")

    outs = [np.asarray(r["out"]).reshape(R, FO) for r in res.results]
    full = np.concatenate([o[:S] for o in outs], axis=0)[:N]
    return full.astype(np.float32)
